# revision 2
# baseline (speedup 1.0000x reference)
"""MultiHeadSSM Trainium2 kernel (8 NeuronCores).

Module: xp = x @ W_in.T; dt = softplus(xp @ W_dt.T + b_dt);
a = exp(dt[...,None] * -exp(log_A)); linear scan s_t = a_t s_{t-1} + xp_t;
y = ys @ W_out.T; returns (y, final_state).

Sharding: 8 shards = batch(4) x T-halves(2). Core c handles b=c//2,
half=c&1 (2048 tokens, all 2048 channels). The cross-half scan dependency
is broken with the pair-scan decomposition: each core computes the local
zero-init scan U and (in log-space) the decay cumprod P = exp(cumsum(A*dt));
an 8KB pair AllGather ships the first half's final state s_in, and
ys = U + P * s_in (s_in masked to 0 on even cores). dt is computed directly
from x via host-precomputed W_dteff = W_dt @ W_in, so the whole pipeline is
uniform SPMD with a single tiny collective.

On-device layout: channels on partitions, time on the free dim. Matmuls run
as float32r (full PE rate, ~1e-4 relative precision). The recurrence runs on
the DVE tensor_tensor_scan instruction.
"""
import sys

sys.path.insert(0, "/opt/trn_rl_repo")

import numpy as np

D = 2048          # d_model
H = 32            # heads
HD = 64           # head_dim
B = 4
T = 4096
NCORES = 8
TC = T // 2       # tokens per core
KB = D // 128     # 16 partition blocks of channels
TT_A = 512        # phase-A time tile
NTA = TC // TT_A  # 4
TT_B = 256        # phase-B time tile
NTB = TC // TT_B  # 8
EW = 512          # phase-B out-proj e-chunk
NE = D // EW      # 4

_cache = {}


def _build():
    import concourse.bass as bass
    import concourse.bacc as bacc
    import concourse.tile as tile
    from concourse import mybir

    f32 = mybir.dt.float32
    f32r = mybir.dt.float32r
    AF = mybir.ActivationFunctionType
    OP = mybir.AluOpType
    ts = bass.ts

    nc = bacc.Bacc("TRN2", target_bir_lowering=False, debug=False, num_devices=NCORES)

    xt_d = nc.dram_tensor("xt", (D, TC), f32r, kind="ExternalInput").ap()
    w_in_d = nc.dram_tensor("w_in_t", (D, D), f32r, kind="ExternalInput").ap()
    w_out_d = nc.dram_tensor("w_out_t", (D, D), f32r, kind="ExternalInput").ap()
    w_dteff_d = nc.dram_tensor("w_dteff_t", (D, H), f32r, kind="ExternalInput").ap()
    sa_d = nc.dram_tensor("sa", (H, D), f32r, kind="ExternalInput").ap()
    bdt_d = nc.dram_tensor("b_dt", (H, 1), f32, kind="ExternalInput").ap()
    cmask_d = nc.dram_tensor("cmask", (128, 1), f32, kind="ExternalInput").ap()
    y_d = nc.dram_tensor("y_out", (TC, D), f32, kind="ExternalOutput").ap()
    s_d = nc.dram_tensor("s_out", (128, KB), f32, kind="ExternalOutput").ap()

    xt_r = xt_d.rearrange("(k p) t -> p k t", p=128)
    w_in_r = w_in_d.rearrange("(k p) e -> p k e", p=128)
    w_out_r = w_out_d.rearrange("(k p) e -> p k e", p=128)
    w_dteff_r = w_dteff_d.rearrange("(k p) h -> p k h", p=128)

    with tile.TileContext(nc) as tc:
        with (
            tc.tile_pool(name="bigpool", bufs=1) as bigpool,
            tc.tile_pool(name="cpool", bufs=1) as cpool,
            tc.tile_pool(name="dram", bufs=1, space="DRAM") as dram,
            tc.tile_pool(name="psdtb", bufs=2, space=bass.MemorySpace.PSUM) as psdtb_pool,
        ):
            u_dram = dram.tile([D, TC], f32)
            cc_in = dram.tile([KB, 128], f32)
            cc_out = dram.tile([2, KB, 128], f32)
            u_r = u_dram[:].rearrange("(j p) t -> p j t", p=128)

            # persistent small tiles
            wdteff = cpool.tile([128, KB, H], f32r)
            sa_sb = cpool.tile([H, D], f32r)
            bdt_sb = cpool.tile([H, 1], f32)
            cmask_sb = cpool.tile([128, 1], f32)
            dt_sb = cpool.tile([H, TC], f32r)
            carry_u = cpool.tile([128, KB], f32)
            carry_s = cpool.tile([128, KB], f32)
            sfin_raw = cpool.tile([128, KB], f32)
            sfin = cpool.tile([128, KB], f32)
            s_stage = cpool.tile([128, KB], f32)
            zeros_sb = cpool.tile([128, TT_B], f32)
            nc.gpsimd.memset(zeros_sb[:], 0.0)

            nc.sync.dma_start(wdteff[:], w_dteff_r[:])
            nc.sync.dma_start(sa_sb[:], sa_d[:])
            nc.sync.dma_start(bdt_sb[:], bdt_d[:])
            nc.sync.dma_start(cmask_sb[:], cmask_d[:])

            # x resident, (tc, k) order so the dt matmuls can start early
            xsb = bigpool.tile([128, KB, TC], f32r, tag="big")
            for tci in range(NTA):
                for k in range(KB):
                    nc.sync.dma_start(
                        xsb[:, k, ts(tci, TT_A)], xt_r[:, k, ts(tci, TT_A)]
                    )

            # ---------------- phase A: dt, in_proj, decay, scan U ----------
            with (
                tc.tile_pool(name="psdt", bufs=2, space=bass.MemorySpace.PSUM) as psdt_pool,
                tc.tile_pool(name="psxp", bufs=3, space=bass.MemorySpace.PSUM) as psxp_pool,
                tc.tile_pool(name="win", bufs=2) as win_pool,
                tc.tile_pool(name="apool", bufs=2) as a_pool,
                tc.tile_pool(name="upool", bufs=2) as u_pool,
                tc.tile_pool(name="spool", bufs=2) as small_pool,
            ):
                # dt_pre = W_dteff @ x ; dt = softplus(dt_pre + b_dt)
                for tci in range(NTA):
                    ps_dt = psdt_pool.tile([H, TT_A], f32)
                    for k in range(KB):
                        nc.tensor.matmul(
                            ps_dt[:],
                            wdteff[:, k, :],
                            xsb[:, k, ts(tci, TT_A)],
                            start=(k == 0),
                            stop=(k == KB - 1),
                        )
                    e_sb = small_pool.tile([H, TT_A], f32)
                    nc.scalar.activation(e_sb[:], ps_dt[:], AF.Exp, bias=bdt_sb[:, 0:1])
                    nc.scalar.activation(
                        dt_sb[:, ts(tci, TT_A)], e_sb[:], AF.Ln, bias=1.0
                    )

                for j in range(KB):
                    wj = win_pool.tile([128, KB, 128], f32r)
                    nc.sync.dma_start(wj[:], w_in_r[:, :, ts(j, 128)])
                    for tci in range(NTA):
                        ps_xp = psxp_pool.tile([128, TT_A], f32)
                        for k in range(KB):
                            nc.tensor.matmul(
                                ps_xp[:],
                                wj[:, k, :],
                                xsb[:, k, ts(tci, TT_A)],
                                start=(k == 0),
                                stop=(k == KB - 1),
                            )
                        ps_dtb = psdtb_pool.tile([128, TT_A], f32, tag="dtb")
                        nc.tensor.matmul(
                            ps_dtb[:],
                            sa_sb[:, ts(j, 128)],
                            dt_sb[:, ts(tci, TT_A)],
                            start=True,
                            stop=True,
                        )
                        a_sb = a_pool.tile([128, TT_A], f32)
                        nc.scalar.activation(a_sb[:], ps_dtb[:], AF.Exp)
                        u_sb = u_pool.tile([128, TT_A], f32)
                        init = 0.0 if tci == 0 else carry_u[:, j : j + 1]
                        nc.vector.tensor_tensor_scan(
                            u_sb[:], a_sb[:], ps_xp[:], init,
                            op0=OP.mult, op1=OP.add,
                        )
                        nc.vector.tensor_copy(
                            carry_u[:, j : j + 1], u_sb[:, TT_A - 1 : TT_A]
                        )
                        nc.sync.dma_start(u_r[:, j, ts(tci, TT_A)], u_sb[:])

            # ---------------- pair handoff of final local states ------------
            for j in range(KB):
                nc.sync.dma_start(cc_in[j, :], carry_u[:, j : j + 1])
            nc.gpsimd.collective_compute(
                "AllGather",
                OP.bypass,
                replica_groups=[[0, 1], [2, 3], [4, 5], [6, 7]],
                ins=[cc_in.opt()],
                outs=[cc_out.opt()],
            )
            for j in range(KB):
                nc.sync.dma_start(sfin_raw[:, j : j + 1], cc_out[0, j, :])
            nc.vector.tensor_scalar_mul(sfin[:], sfin_raw[:], cmask_sb[:, 0:1])

            # ---------------- phase B: correction + out_proj ----------------
            wout = bigpool.tile([128, KB, D], f32r, tag="big")
            for k in range(KB):
                nc.sync.dma_start(wout[:, k, :], w_out_r[:, k, :])

            with (
                tc.tile_pool(name="psy", bufs=3, space=bass.MemorySpace.PSUM) as psy_pool,
                tc.tile_pool(name="uld", bufs=3) as uld_pool,
                tc.tile_pool(name="ssb", bufs=2) as ssb_pool,
                tc.tile_pool(name="ppool", bufs=2) as p_pool,
                tc.tile_pool(name="yspool", bufs=1) as ys_pool,
                tc.tile_pool(name="ystage", bufs=3) as ystage_pool,
            ):
                for tci in range(NTB):
                    ys = ys_pool.tile([128, KB, TT_B], f32r, tag="ys")
                    for j in range(KB):
                        u_ld = uld_pool.tile([128, TT_B], f32)
                        nc.sync.dma_start(u_ld[:], u_r[:, j, ts(tci, TT_B)])
                        ps_dtb = psdtb_pool.tile([128, TT_B], f32, tag="dtb")
                        nc.tensor.matmul(
                            ps_dtb[:],
                            sa_sb[:, ts(j, 128)],
                            dt_sb[:, ts(tci, TT_B)],
                            start=True,
                            stop=True,
                        )
                        s_sb = ssb_pool.tile([128, TT_B], f32)
                        init = 0.0 if tci == 0 else carry_s[:, j : j + 1]
                        nc.vector.tensor_tensor_scan(
                            s_sb[:], ps_dtb[:], zeros_sb[:], init,
                            op0=OP.add, op1=OP.add,
                        )
                        nc.vector.tensor_copy(
                            carry_s[:, j : j + 1], s_sb[:, TT_B - 1 : TT_B]
                        )
                        p_sb = p_pool.tile([128, TT_B], f32)
                        nc.scalar.activation(p_sb[:], s_sb[:], AF.Exp)
                        nc.vector.scalar_tensor_tensor(
                            ys[:, j, :], p_sb[:], sfin[:, j : j + 1], u_ld[:],
                            op0=OP.mult, op1=OP.add,
                        )
                    for m in range(TT_B // 128):
                        for ne in range(NE):
                            ps_y = psy_pool.tile([128, EW], f32)
                            for j in range(KB):
                                nc.tensor.matmul(
                                    ps_y[:],
                                    ys[:, j, m * 128 : (m + 1) * 128],
                                    wout[:, j, ts(ne, EW)],
                                    start=(j == 0),
                                    stop=(j == KB - 1),
                                )
                            y_st = ystage_pool.tile([128, EW], f32)
                            nc.scalar.copy(y_st[:], ps_y[:])
                            row0 = tci * TT_B + m * 128
                            nc.sync.dma_start(
                                y_d[row0 : row0 + 128, ts(ne, EW)], y_st[:]
                            )
                    if tci == NTB - 1:
                        for j in range(KB):
                            nc.vector.tensor_copy(
                                s_stage[:, j : j + 1],
                                ys[:, j, TT_B - 1 : TT_B].bitcast(f32),
                            )
                        nc.sync.dma_start(s_d[:], s_stage[:])

    nc.compile()
    return nc


def _get_nc():
    if "nc" not in _cache:
        _cache["nc"] = _build()
    return _cache["nc"]


def kernel(x, W_in, W_out, log_A, W_dt, b_dt):
    from concourse.bass_utils import run_bass_kernel_spmd

    x = np.asarray(x, dtype=np.float32)
    W_in = np.asarray(W_in, dtype=np.float32)
    W_out = np.asarray(W_out, dtype=np.float32)
    log_A = np.asarray(log_A, dtype=np.float32)
    W_dt = np.asarray(W_dt, dtype=np.float32)
    b_dt = np.asarray(b_dt, dtype=np.float32)

    nc = _get_nc()

    W_inT = np.ascontiguousarray(W_in.T)
    W_outT = np.ascontiguousarray(W_out.T)
    W_dteff = (W_dt.astype(np.float64) @ W_in.astype(np.float64)).astype(np.float32)
    W_dteffT = np.ascontiguousarray(W_dteff.T)
    A_flat = (-np.exp(log_A.astype(np.float64))).astype(np.float32).reshape(D)
    SA = np.zeros((H, D), dtype=np.float32)
    SA[np.arange(D) // HD, np.arange(D)] = A_flat
    bdt_col = np.ascontiguousarray(b_dt.reshape(H, 1))

    in_maps = []
    for c in range(NCORES):
        b, half = c >> 1, c & 1
        xt = np.ascontiguousarray(x[b, half * TC : (half + 1) * TC, :].T)
        in_maps.append(
            {
                "xt": xt,
                "w_in_t": W_inT,
                "w_out_t": W_outT,
                "w_dteff_t": W_dteffT,
                "sa": SA,
                "b_dt": bdt_col,
                "cmask": np.full((128, 1), float(half), dtype=np.float32),
            }
        )

    res = run_bass_kernel_spmd(nc, in_maps, core_ids=list(range(NCORES)))
    _cache["last_res"] = res

    y = np.empty((B, T, D), dtype=np.float32)
    s = np.empty((B, H, HD), dtype=np.float32)
    for c in range(NCORES):
        r = res.results[c]
        b, half = c >> 1, c & 1
        y[b, half * TC : (half + 1) * TC, :] = r["y_out"]
        if half == 1:
            s[b] = r["s_out"].T.reshape(H, HD)
    return y, s
